# revision 1
# baseline (speedup 1.0000x reference)
"""Fused 2-layer peephole ConvLSTM for TRN2 (Bass/Tile), 8-core SPMD.

Problem: x[B=16, T=25, CIN=3, W=4096] -> y[B, T, HID=32, W]; two stacked
ConvLSTM layers (k=3 SAME conv over W, peephole connections), zero-init
states, scanned over T.

Sharding: data-parallel over batch. B=16 -> 2 batches per core on 8 cores;
weights replicated; no collectives. The full recurrence runs on-chip: only
x is read from HBM and only y (layer-1 hidden states) is written back.

Per-core layout ("q-packing"): per-step logical tensors [32ch, 2*W] are
stored as [128, WH] tiles (WH = W/2), partition block 32q:32q+32 holding
quarter q = (batch q>>1, w-half q&1). Every elementwise/activation op over
a step then runs as a single [128, WH] instruction (engine cost tracks the
free-dim length, independent of partition count), and the PE runs the four
quarters' conv matmuls on four different row-groups via tile_position.

Per step, per layer:
  PE : per quarter q (row-group q): 3 h-tap matmuls (K=32, M=128) [+ 3
       x/y-source tap matmuls] + peephole-i/f matmul (wci/wcf padded to
       M=128), accumulated in a PSUM tile [128, WH] with gate rows
       i 0:32 | f 32:64 | o 64:96 | g 96:128.
  ACT: per quarter: sigmoid(i), sigmoid(f), tanh(g), Identity+bias(o),
       each [32, WH] PSUM -> q-packed SBUF.
  DVE: m1 = s_f*c ; m2 = s_i*tanh_g ; c' = m1+m2     (single [128, WH] ops)
  PE : peephole-o: 4 diagonal tile_position matmuls wco.T @ c' -> PSUM
  DVE: o_pre += peep_o ; ACT: s_o = sigmoid(o_pre), t_c = tanh(c')
  DVE: h' = s_o * t_c -> h tile core cols + 4 halo-column copies
  DMA: (layer 1) h' -> y HBM
"""

import sys

for _p in ("/opt/trn_rl_repo",):
    if _p not in sys.path:
        sys.path.insert(0, _p)

from contextlib import ExitStack

import numpy as np

import concourse.bass as bass  # noqa: F401  (bass types used via tile/bacc)
import concourse.tile as tile
from concourse import bacc, mybir
from concourse.bass_utils import run_bass_kernel_spmd

F32 = mybir.dt.float32
AF = mybir.ActivationFunctionType

B, T, CIN, HID, W = 16, 25, 3, 32, 4096
KTAP = 3
N_CORES = 8
B_SHARD = B // N_CORES  # 2
NQ = 4  # quarters: (batch 0/1) x (w-half 0/1)

WEIGHT_SHAPES = dict(
    w0h=(128, 384),
    w0x=(128, 384),
    wp0=(128, 128),
    wpo0=(128, 32),
    b0=(128, 1),
    w1h=(128, 768),
    wp1=(128, 128),
    wpo1=(128, 32),
    b1=(128, 1),
)


def _pack_weights(conv_w0, conv_b0, wci0, wcf0, wco0, conv_w1, conv_b1, wci1, wcf1, wco1):
    """Pack reference weights into the SBUF layouts the kernel expects.

    Each [K, M] block is replicated into the 4 partition quarters (the PE
    row-group q streams its rhs, and loads its lhsT, from partitions 32q).
    """

    def rep4(block):
        out = np.zeros((128, block.shape[1]), np.float32)
        for q in range(NQ):
            out[32 * q : 32 * q + block.shape[0]] = block
        return out

    def taps(conv_w, in_lo, in_hi):
        # [k_in, 128out] per tap, taps concatenated on axis 1
        return np.concatenate(
            [np.asarray(conv_w[:, in_lo:in_hi, d]).T for d in range(KTAP)], axis=1
        ).astype(np.float32)

    def peep_if(wci, wcf):
        # lhsT [32, 128]: out cols 0:32 -> i rows, 32:64 -> f rows, rest 0
        blk = np.zeros((HID, 128), np.float32)
        blk[:, 0:HID] = np.asarray(wci).T
        blk[:, HID : 2 * HID] = np.asarray(wcf).T
        return blk

    return dict(
        w0h=rep4(taps(conv_w0, CIN, CIN + HID)),
        w0x=rep4(taps(conv_w0, 0, CIN)),
        wp0=rep4(peep_if(wci0, wcf0)),
        wpo0=rep4(np.asarray(wco0).T.astype(np.float32)),
        b0=np.asarray(conv_b0, np.float32).reshape(128, 1),
        w1h=rep4(
            np.concatenate([taps(conv_w1, 0, HID), taps(conv_w1, HID, 2 * HID)], 1)
        ),
        wp1=rep4(peep_if(wci1, wcf1)),
        wpo1=rep4(np.asarray(wco1).T.astype(np.float32)),
        b1=np.asarray(conv_b1, np.float32).reshape(128, 1),
    )


def _build_kernel():
    WH = W // 2
    MMN = 512  # fp32 matmul free-dim cap (one PSUM bank)
    n_chunks = WH // MMN

    nc = bacc.Bacc("TRN2", target_bir_lowering=False, debug=False)

    x_d = nc.dram_tensor("x", [B_SHARD, T, CIN, W], F32, kind="ExternalInput")
    y_d = nc.dram_tensor("y", [B_SHARD, T, HID, W], F32, kind="ExternalOutput")
    w_d = {
        name: nc.dram_tensor(name, list(shape), F32, kind="ExternalInput")
        for name, shape in WEIGHT_SHAPES.items()
    }
    x_ap = x_d.ap()
    y_ap = y_d.ap()

    with tile.TileContext(nc) as tc, ExitStack() as ctx:
        const = ctx.enter_context(tc.tile_pool(name="const", bufs=1))
        xpool = ctx.enter_context(tc.tile_pool(name="xp", bufs=3))
        gates = ctx.enter_context(tc.tile_pool(name="gates", bufs=2))
        state = ctx.enter_context(tc.tile_pool(name="state", bufs=1))
        psum = ctx.enter_context(tc.tile_pool(name="psum", bufs=2, space="PSUM"))

        wt = {}
        for name, shape in WEIGHT_SHAPES.items():
            wtile = const.tile(list(shape), F32, name=f"wt_{name}", tag=f"wt_{name}")
            nc.sync.dma_start(wtile[:], w_d[name].ap()[:, :])
            wt[name] = wtile

        # persistent state, ping-pong buffers; h tiles carry 1 halo col/side
        h0 = [state.tile([128, WH + 2], F32, tag=f"h0_{i}", name=f"h0_{i}") for i in range(2)]
        h1 = [state.tile([128, WH + 2], F32, tag=f"h1_{i}", name=f"h1_{i}") for i in range(2)]
        c0 = [state.tile([128, WH], F32, tag=f"c0_{i}", name=f"c0_{i}") for i in range(2)]
        c1 = [state.tile([128, WH], F32, tag=f"c1_{i}", name=f"c1_{i}") for i in range(2)]
        for tl in (*h0, *h1, *c0, *c1):
            nc.vector.memset(tl[:], 0.0)

        def halo_fix(h):
            nc.vector.tensor_copy(h[32:64, 0:1], h[0:32, WH : WH + 1])
            nc.vector.tensor_copy(h[0:32, WH + 1 : WH + 2], h[32:64, 1:2])
            nc.vector.tensor_copy(h[96:128, 0:1], h[64:96, WH : WH + 1])
            nc.vector.tensor_copy(h[64:96, WH + 1 : WH + 2], h[96:128, 1:2])

        def load_x(t):
            xt = xpool.tile([128, WH + 2], F32, tag="xt", name="xt")
            for q in range(NQ):
                b, half = q >> 1, q & 1
                w0 = half * WH
                lo, hi = w0 - 1, w0 + WH + 1
                slo, shi = max(lo, 0), min(hi, W)
                dlo = slo - lo
                dhi = (WH + 2) - (hi - shi)
                rows = slice(32 * q, 32 * q + CIN)
                nc.sync.dma_start(xt[rows, dlo:dhi], x_ap[b, t, 0:CIN, slo:shi])
                if dlo > 0:
                    nc.vector.memset(xt[rows, 0:dlo], 0.0)
                if dhi < WH + 2:
                    nc.vector.memset(xt[rows, dhi : WH + 2], 0.0)
            return xt

        def step_layer(lyr, xt, h_prev, h_next, c_prev, c_next, y_src):
            w_h = wt["w0h"] if lyr == 0 else wt["w1h"]
            w_p = wt["wp0"] if lyr == 0 else wt["wp1"]
            w_po = wt["wpo0"] if lyr == 0 else wt["wpo1"]
            b_t = wt["b0"] if lyr == 0 else wt["b1"]

            s_i = gates.tile([128, WH], F32, tag="s_i", name="s_i")
            s_f = gates.tile([128, WH], F32, tag="s_f", name="s_f")
            t_g = gates.tile([128, WH], F32, tag="t_g", name="t_g")
            o_pre = gates.tile([128, WH], F32, tag="o_pre", name="o_pre")
            s_o = gates.tile([128, WH], F32, tag="s_o", name="s_o", bufs=1)
            t_c = gates.tile([128, WH], F32, tag="t_c", name="t_c", bufs=1)
            m1 = gates.tile([128, WH], F32, tag="m1", name="m1", bufs=1)
            m2 = gates.tile([128, WH], F32, tag="m2", name="m2", bufs=1)

            for q in range(NQ):
                rq = slice(32 * q, 32 * q + 32)
                pq = psum.tile([128, WH], F32, tag="P", name="pq")
                for ch in range(n_chunks):
                    c_lo, c_hi = ch * MMN, (ch + 1) * MMN
                    cs = slice(c_lo, c_hi)
                    if lyr == 0:
                        srcs = [(xt, wt["w0x"], 0, CIN), (h_prev, w_h, 0, HID)]
                    else:
                        srcs = [(y_src, w_h, 0, HID), (h_prev, w_h, 384, HID)]
                    first = True
                    for src, w_src, wcol, kdim in srcs:
                        for d in range(KTAP):
                            nc.tensor.matmul(
                                pq[:, cs],
                                w_src[32 * q : 32 * q + kdim,
                                      wcol + 128 * d : wcol + 128 * d + 128],
                                src[32 * q : 32 * q + kdim, c_lo + d : c_hi + d],
                                start=first,
                                stop=False,
                                tile_position=(32 * q, 0),
                            )
                            first = False
                    nc.tensor.matmul(
                        pq[:, cs],
                        w_p[rq, :],
                        c_prev[rq, cs],
                        start=False,
                        stop=True,
                        tile_position=(32 * q, 0),
                    )

                nc.scalar.activation(s_i[rq, :], pq[0:32, :], AF.Sigmoid, bias=b_t[0:32, :])
                nc.scalar.activation(s_f[rq, :], pq[32:64, :], AF.Sigmoid, bias=b_t[32:64, :])
                nc.scalar.activation(t_g[rq, :], pq[96:128, :], AF.Tanh, bias=b_t[96:128, :])
                nc.scalar.activation(o_pre[rq, :], pq[64:96, :], AF.Identity, bias=b_t[64:96, :])

            nc.vector.tensor_mul(m1[:], s_f[:], c_prev[:])
            nc.vector.tensor_mul(m2[:], s_i[:], t_g[:])
            nc.vector.tensor_add(c_next[:], m1[:], m2[:])

            # peephole o (diagonal tiles, q-packed out, shares "P" slots)
            po = psum.tile([128, WH], F32, tag="P", name="po")
            for q in range(NQ):
                rq = slice(32 * q, 32 * q + 32)
                for ch in range(n_chunks):
                    cs = slice(ch * MMN, (ch + 1) * MMN)
                    nc.tensor.matmul(
                        po[rq, cs],
                        w_po[rq, :],
                        c_next[rq, cs],
                        start=True,
                        stop=True,
                        tile_position=(32 * q, 32 * q),
                    )
            nc.vector.tensor_add(o_pre[:], o_pre[:], po[:])

            nc.scalar.activation(s_o[:], o_pre[:], AF.Sigmoid)
            nc.scalar.activation(t_c[:], c_next[:], AF.Tanh)
            nc.vector.tensor_mul(h_next[:, 1 : WH + 1], s_o[:], t_c[:])
            halo_fix(h_next)

        for t in range(T):
            cur, nxt = t % 2, (t + 1) % 2
            xt = load_x(t)
            step_layer(0, xt, h0[cur], h0[nxt], c0[cur], c0[nxt], None)
            step_layer(1, None, h1[cur], h1[nxt], c1[cur], c1[nxt], h0[nxt])
            for q in range(NQ):
                b, half = q >> 1, q & 1
                w0 = half * WH
                nc.sync.dma_start(
                    y_ap[b, t, 0:HID, w0 : w0 + WH],
                    h1[nxt][32 * q : 32 * q + 32, 1 : WH + 1],
                )

    nc.compile()
    return nc


_NC_CACHE = None


def _get_nc():
    global _NC_CACHE
    if _NC_CACHE is None:
        _NC_CACHE = _build_kernel()
    return _NC_CACHE


def kernel(x, conv_w0, conv_b0, wci0, wcf0, wco0,
           conv_w1, conv_b1, wci1, wcf1, wco1):
    x = np.ascontiguousarray(np.asarray(x, np.float32))
    packed = _pack_weights(conv_w0, conv_b0, wci0, wcf0, wco0,
                           conv_w1, conv_b1, wci1, wcf1, wco1)
    nc = _get_nc()
    in_maps = []
    for core in range(N_CORES):
        m = {"x": np.ascontiguousarray(x[B_SHARD * core : B_SHARD * (core + 1)])}
        m.update(packed)
        in_maps.append(m)
    res = run_bass_kernel_spmd(nc, in_maps, core_ids=list(range(N_CORES)))
    return np.concatenate([r["y"] for r in res.results], axis=0)



# revision 6
# speedup vs baseline: 1402.5429x; 1402.5429x over previous
"""Fused 2-layer peephole ConvLSTM for TRN2 (Bass/Tile), 8-core SPMD.

Problem: x[B=16, T=25, CIN=3, W=4096] -> y[B, T, HID=32, W]; two stacked
ConvLSTM layers (k=3 SAME conv over W, peephole connections), zero-init
states, scanned over T.

Sharding: data-parallel over batch. B=16 -> 2 batches per core on 8 cores;
weights replicated; no collectives. The full recurrence runs on-chip: only
x is read from HBM and only y (layer-1 hidden states) is written back.

Per-core layout ("q-packing"): per-step logical tensors [32ch, 2*W] are
stored as [128, WH] tiles (WH = W/2), partition block 32q:32q+32 holding
quarter q = (batch q>>1, w-half q&1). Every elementwise/activation op over
a step then runs as a single [128, WH] instruction (engine cost tracks the
free-dim length, independent of partition count), and the PE runs the four
quarters' conv matmuls on four different row-groups via tile_position.

Per step, per layer:
  PE : per quarter q (row-group q): 3 h-tap matmuls (K=32, M=128) [+ 3
       x/y-source tap matmuls] + peephole-i/f matmul (wci/wcf padded to
       M=128), accumulated in a PSUM tile [128, WH] with gate rows
       i 0:32 | f 32:64 | o 64:96 | g 96:128.
  ACT: per quarter: sigmoid(i), sigmoid(f), tanh(g), Identity+bias(o),
       each [32, WH] PSUM -> q-packed SBUF.
  DVE: m1 = s_f*c ; m2 = s_i*tanh_g ; c' = m1+m2     (single [128, WH] ops)
  PE : peephole-o: 4 diagonal tile_position matmuls wco.T @ c' -> PSUM
  DVE: o_pre += peep_o ; ACT: s_o = sigmoid(o_pre), t_c = tanh(c')
  DVE: h' = s_o * t_c -> h tile core cols + 4 halo-column copies
  DMA: (layer 1) h' -> y HBM
"""

import sys

for _p in ("/opt/trn_rl_repo",):
    if _p not in sys.path:
        sys.path.insert(0, _p)

from contextlib import ExitStack

import numpy as np

import concourse.bass as bass  # noqa: F401  (bass types used via tile/bacc)
import concourse.tile as tile
from concourse import bacc, mybir
from concourse.bass_utils import run_bass_kernel_spmd

F32 = mybir.dt.float32
BF16 = mybir.dt.bfloat16
AF = mybir.ActivationFunctionType

import ml_dtypes

BF = ml_dtypes.bfloat16

B, T, CIN, HID, W = 16, 25, 3, 32, 4096
KTAP = 3
N_CORES = 8
B_SHARD = B // N_CORES  # 2
NQ = 4  # quarters: (batch 0/1) x (w-half 0/1)

WEIGHT_SHAPES = dict(
    w0h=(128, 384),
    w0x=(128, 128),
    wp0=(128, 128),
    wpo0=(128, 32),
    b0=(128, 1),
    w1h=(128, 768),
    wp1=(128, 128),
    wpo1=(128, 32),
    b1=(128, 1),
)


def _pack_weights(conv_w0, conv_b0, wci0, wcf0, wco0, conv_w1, conv_b1, wci1, wcf1, wco1):
    """Pack reference weights into the SBUF layouts the kernel expects.

    Each [K, M] block is replicated into the 4 partition quarters (the PE
    row-group q streams its rhs, and loads its lhsT, from partitions 32q).
    """

    def rep4(block):
        out = np.zeros((128, block.shape[1]), np.float32)
        for q in range(NQ):
            out[32 * q : 32 * q + block.shape[0]] = block
        return out

    def taps(conv_w, in_lo, in_hi):
        # [k_in, 128out] per tap, taps concatenated on axis 1
        return np.concatenate(
            [np.asarray(conv_w[:, in_lo:in_hi, d]).T for d in range(KTAP)], axis=1
        ).astype(np.float32)

    def peep_if(wci, wcf):
        # lhsT [32, 128]: out cols 0:32 -> i rows, 32:64 -> f rows, rest 0
        blk = np.zeros((HID, 128), np.float32)
        blk[:, 0:HID] = np.asarray(wci).T
        blk[:, HID : 2 * HID] = np.asarray(wcf).T
        return blk

    # x im2col block: row 3*d + c = tap d, channel c -> [9, 128]
    xblk = np.concatenate(
        [np.asarray(conv_w0[:, 0:CIN, d]).T for d in range(KTAP)], axis=0
    ).astype(np.float32)
    d = dict(
        w0h=rep4(taps(conv_w0, CIN, CIN + HID)),
        w0x=rep4(xblk),
        wp0=rep4(peep_if(wci0, wcf0)),
        wpo0=rep4(np.asarray(wco0).T.astype(np.float32)),
        b0=np.asarray(conv_b0, np.float32).reshape(128, 1),
        w1h=rep4(
            np.concatenate([taps(conv_w1, 0, HID), taps(conv_w1, HID, 2 * HID)], 1)
        ),
        wp1=rep4(peep_if(wci1, wcf1)),
        wpo1=rep4(np.asarray(wco1).T.astype(np.float32)),
        b1=np.asarray(conv_b1, np.float32).reshape(128, 1),
    )
    return {k: (v if k.startswith("b") else v.astype(BF)) for k, v in d.items()}


def _build_kernel():
    WH = W // 2
    MMN = 512  # fp32 matmul free-dim cap (one PSUM bank)
    n_chunks = WH // MMN

    nc = bacc.Bacc("TRN2", target_bir_lowering=False, debug=False)

    x_d = nc.dram_tensor("x", [B_SHARD, T, CIN, W], BF16, kind="ExternalInput")
    y_d = nc.dram_tensor("y", [B_SHARD, T, HID, W], BF16, kind="ExternalOutput")
    w_d = {
        name: nc.dram_tensor(
            name, list(shape), F32 if name.startswith("b") else BF16,
            kind="ExternalInput")
        for name, shape in WEIGHT_SHAPES.items()
    }
    x_ap = x_d.ap()
    y_ap = y_d.ap()

    with tile.TileContext(nc) as tc, ExitStack() as ctx:
        const = ctx.enter_context(tc.tile_pool(name="const", bufs=1))
        xpool = ctx.enter_context(tc.tile_pool(name="xp", bufs=3))
        gates = ctx.enter_context(tc.tile_pool(name="gates", bufs=2))
        state = ctx.enter_context(tc.tile_pool(name="state", bufs=1))
        psum = ctx.enter_context(tc.tile_pool(name="psum", bufs=2, space="PSUM"))

        wt = {}
        for name, shape in WEIGHT_SHAPES.items():
            wtile = const.tile(
                list(shape), F32 if name.startswith("b") else BF16,
                name=f"wt_{name}", tag=f"wt_{name}")
            nc.sync.dma_start(wtile[:], w_d[name].ap()[:, :])
            wt[name] = wtile

        # persistent state, ping-pong buffers; h tiles carry 1 halo col/side
        h0 = [state.tile([128, WH + 2], BF16, tag=f"h0_{i}", name=f"h0_{i}") for i in range(2)]
        h1 = [state.tile([128, WH + 2], BF16, tag=f"h1_{i}", name=f"h1_{i}") for i in range(2)]
        c0 = [state.tile([128, WH], F32, tag=f"c0_{i}", name=f"c0_{i}") for i in range(2)]
        c1 = [state.tile([128, WH], F32, tag=f"c1_{i}", name=f"c1_{i}") for i in range(2)]
        # bf16 copies of c for PE rhs (peephole matmuls); fp32 master for DVE/ACT
        cb0 = [state.tile([128, WH], BF16, tag=f"cb0_{i}", name=f"cb0_{i}") for i in range(2)]
        cb1 = [state.tile([128, WH], BF16, tag=f"cb1_{i}", name=f"cb1_{i}") for i in range(2)]
        for tl in (*h0, *h1, *c0, *c1, *cb0, *cb1):
            nc.vector.memset(tl[:], 0.0)

        def halo_fix(h):
            nc.vector.tensor_copy(h[32:64, 0:1], h[0:32, WH : WH + 1])
            nc.vector.tensor_copy(h[0:32, WH + 1 : WH + 2], h[32:64, 1:2])
            nc.vector.tensor_copy(h[96:128, 0:1], h[64:96, WH : WH + 1])
            nc.vector.tensor_copy(h[64:96, WH + 1 : WH + 2], h[96:128, 1:2])

        def load_x(t):
            # im2col over taps: rows 32q + 3d + c = x[b, t, c, w + d - 1]
            # (one K=9 matmul per chunk-quarter instead of 3 K=3 matmuls)
            xt = xpool.tile([128, WH], BF16, tag="xt", name="xt")
            for q in range(NQ):
                b, half = q >> 1, q & 1
                w0 = half * WH
                for d in range(KTAP):
                    lo = w0 + d - 1
                    hi = lo + WH
                    slo, shi = max(lo, 0), min(hi, W)
                    dlo = slo - lo
                    dhi = WH - (hi - shi)
                    rows = slice(32 * q + 3 * d, 32 * q + 3 * d + CIN)
                    nc.sync.dma_start(xt[rows, dlo:dhi], x_ap[b, t, 0:CIN, slo:shi])
                    if dlo > 0:
                        nc.vector.memset(xt[rows, 0:dlo], 0.0)
                    if dhi < WH:
                        nc.vector.memset(xt[rows, dhi:WH], 0.0)
            return xt

        def step_layer(lyr, xt, h_prev, h_next, c_prev, c_next, cb_prev, cb_next, y_src):
            w_h = wt["w0h"] if lyr == 0 else wt["w1h"]
            w_p = wt["wp0"] if lyr == 0 else wt["wp1"]
            w_po = wt["wpo0"] if lyr == 0 else wt["wpo1"]
            b_t = wt["b0"] if lyr == 0 else wt["b1"]

            s_i = gates.tile([128, WH], F32, tag="s_i", name="s_i")
            s_f = gates.tile([128, WH], F32, tag="s_f", name="s_f")
            t_g = gates.tile([128, WH], F32, tag="t_g", name="t_g")
            o_pre = gates.tile([128, WH], F32, tag="o_pre", name="o_pre")
            s_o = gates.tile([128, WH], F32, tag="s_o", name="s_o", bufs=1)
            t_c = gates.tile([128, WH], F32, tag="t_c", name="t_c", bufs=1)
            m1 = gates.tile([128, WH], F32, tag="m1", name="m1", bufs=1)
            m2 = gates.tile([128, WH], F32, tag="m2", name="m2", bufs=1)

            for q in range(NQ):
                rq = slice(32 * q, 32 * q + 32)
                pq = psum.tile([128, WH], F32, tag="P", name="pq")
                for ch in range(n_chunks):
                    c_lo, c_hi = ch * MMN, (ch + 1) * MMN
                    cs = slice(c_lo, c_hi)
                    first = True
                    if lyr == 0:
                        # x: single K=9 im2col matmul (taps pre-shifted by DMA)
                        nc.tensor.matmul(
                            pq[:, cs],
                            wt["w0x"][32 * q : 32 * q + 3 * KTAP, 0:128],
                            xt[32 * q : 32 * q + 3 * KTAP, c_lo:c_hi],
                            start=True,
                            stop=False,
                            tile_position=(32 * q, 0),
                        )
                        first = False
                        srcs = [(h_prev, w_h, 0, HID)]
                    else:
                        srcs = [(y_src, w_h, 0, HID), (h_prev, w_h, 384, HID)]
                    for src, w_src, wcol, kdim in srcs:
                        for d in range(KTAP):
                            nc.tensor.matmul(
                                pq[:, cs],
                                w_src[32 * q : 32 * q + kdim,
                                      wcol + 128 * d : wcol + 128 * d + 128],
                                src[32 * q : 32 * q + kdim, c_lo + d : c_hi + d],
                                start=first,
                                stop=False,
                                tile_position=(32 * q, 0),
                            )
                            first = False
                    nc.tensor.matmul(
                        pq[:, cs],
                        w_p[rq, :],
                        cb_prev[rq, cs],
                        start=False,
                        stop=True,
                        tile_position=(32 * q, 0),
                    )

                nc.scalar.activation(s_i[rq, :], pq[0:32, :], AF.Sigmoid, bias=b_t[0:32, :])
                nc.scalar.activation(s_f[rq, :], pq[32:64, :], AF.Sigmoid, bias=b_t[32:64, :])
                nc.scalar.activation(t_g[rq, :], pq[96:128, :], AF.Tanh, bias=b_t[96:128, :])
                nc.scalar.activation(o_pre[rq, :], pq[64:96, :], AF.Identity, bias=b_t[64:96, :])

            nc.vector.tensor_mul(m1[:], s_f[:], c_prev[:])
            nc.vector.tensor_mul(m2[:], s_i[:], t_g[:])
            nc.vector.tensor_add(c_next[:], m1[:], m2[:])
            nc.vector.tensor_copy(cb_next[:], c_next[:])

            # peephole o (diagonal tiles, q-packed out, shares "P" slots)
            po = psum.tile([128, WH], F32, tag="P", name="po")
            for q in range(NQ):
                rq = slice(32 * q, 32 * q + 32)
                for ch in range(n_chunks):
                    cs = slice(ch * MMN, (ch + 1) * MMN)
                    nc.tensor.matmul(
                        po[rq, cs],
                        w_po[rq, :],
                        cb_next[rq, cs],
                        start=True,
                        stop=True,
                        tile_position=(32 * q, 32 * q),
                    )
            nc.vector.tensor_add(o_pre[:], o_pre[:], po[:])

            nc.scalar.activation(s_o[:], o_pre[:], AF.Sigmoid)
            nc.scalar.activation(t_c[:], c_next[:], AF.Tanh)
            nc.vector.tensor_mul(h_next[:, 1 : WH + 1], s_o[:], t_c[:])
            halo_fix(h_next)

        for t in range(T):
            cur, nxt = t % 2, (t + 1) % 2
            xt = load_x(t)
            step_layer(0, xt, h0[cur], h0[nxt], c0[cur], c0[nxt], cb0[cur], cb0[nxt], None)
            step_layer(1, None, h1[cur], h1[nxt], c1[cur], c1[nxt], cb1[cur], cb1[nxt], h0[nxt])
            for q in range(NQ):
                b, half = q >> 1, q & 1
                w0 = half * WH
                nc.sync.dma_start(
                    y_ap[b, t, 0:HID, w0 : w0 + WH],
                    h1[nxt][32 * q : 32 * q + 32, 1 : WH + 1],
                )

    nc.compile()
    return nc


_NC_CACHE = None


def _get_nc():
    global _NC_CACHE
    if _NC_CACHE is None:
        _NC_CACHE = _build_kernel()
    return _NC_CACHE


def kernel(x, conv_w0, conv_b0, wci0, wcf0, wco0,
           conv_w1, conv_b1, wci1, wcf1, wco1):
    x = np.ascontiguousarray(np.asarray(x, np.float32).astype(BF))
    packed = _pack_weights(conv_w0, conv_b0, wci0, wcf0, wco0,
                           conv_w1, conv_b1, wci1, wcf1, wco1)
    nc = _get_nc()
    in_maps = []
    for core in range(N_CORES):
        m = {"x": np.ascontiguousarray(x[B_SHARD * core : B_SHARD * (core + 1)])}
        m.update(packed)
        in_maps.append(m)
    res = run_bass_kernel_spmd(nc, in_maps, core_ids=list(range(N_CORES)))
    return np.concatenate(
        [np.asarray(r["y"]).astype(np.float32) for r in res.results], axis=0)



# revision 7
# speedup vs baseline: 1442.0722x; 1.0282x over previous
"""Fused 2-layer peephole ConvLSTM for TRN2 (Bass/Tile), 8-core SPMD.

Problem: x[B=16, T=25, CIN=3, W=4096] -> y[B, T, HID=32, W]; two stacked
ConvLSTM layers (k=3 SAME conv over W, peephole connections), zero-init
states, scanned over T.

Sharding: data-parallel over batch. B=16 -> 2 batches per core on 8 cores;
weights replicated; no collectives. The full recurrence runs on-chip: only
x is read from HBM and only y (layer-1 hidden states) is written back.

Per-core layout ("q-packing"): per-step logical tensors [32ch, 2*W] are
stored as [128, WH] tiles (WH = W/2), partition block 32q:32q+32 holding
quarter q = (batch q>>1, w-half q&1). Every elementwise/activation op over
a step then runs as a single [128, WH] instruction (engine cost tracks the
free-dim length, independent of partition count), and the PE runs the four
quarters' conv matmuls on four different row-groups via tile_position.

Per step, per layer:
  PE : per quarter q (row-group q): 3 h-tap matmuls (K=32, M=128) [+ 3
       x/y-source tap matmuls] + peephole-i/f matmul (wci/wcf padded to
       M=128), accumulated in a PSUM tile [128, WH] with gate rows
       i 0:32 | f 32:64 | o 64:96 | g 96:128.
  ACT: per quarter: sigmoid(i), sigmoid(f), tanh(g), Identity+bias(o),
       each [32, WH] PSUM -> q-packed SBUF.
  DVE: m1 = s_f*c ; m2 = s_i*tanh_g ; c' = m1+m2     (single [128, WH] ops)
  PE : peephole-o: 4 diagonal tile_position matmuls wco.T @ c' -> PSUM
  DVE: o_pre += peep_o ; ACT: s_o = sigmoid(o_pre), t_c = tanh(c')
  DVE: h' = s_o * t_c -> h tile core cols + 4 halo-column copies
  DMA: (layer 1) h' -> y HBM
"""

import sys

for _p in ("/opt/trn_rl_repo",):
    if _p not in sys.path:
        sys.path.insert(0, _p)

from contextlib import ExitStack

import numpy as np

import concourse.bass as bass  # noqa: F401  (bass types used via tile/bacc)
import concourse.tile as tile
from concourse import bacc, mybir
from concourse.bass_utils import run_bass_kernel_spmd

F32 = mybir.dt.float32
BF16 = mybir.dt.bfloat16
AF = mybir.ActivationFunctionType

import ml_dtypes

BF = ml_dtypes.bfloat16

B, T, CIN, HID, W = 16, 25, 3, 32, 4096
KTAP = 3
N_CORES = 8
B_SHARD = B // N_CORES  # 2
NQ = 4  # quarters: (batch 0/1) x (w-half 0/1)

WEIGHT_SHAPES = dict(
    w0h=(128, 384),
    w0x=(128, 128),
    wp0=(128, 128),
    wpo0=(128, 32),
    b0=(128, 1),
    w1h=(128, 768),
    wp1=(128, 128),
    wpo1=(128, 32),
    b1=(128, 1),
)


def _pack_weights(conv_w0, conv_b0, wci0, wcf0, wco0, conv_w1, conv_b1, wci1, wcf1, wco1):
    """Pack reference weights into the SBUF layouts the kernel expects.

    Each [K, M] block is replicated into the 4 partition quarters (the PE
    row-group q streams its rhs, and loads its lhsT, from partitions 32q).
    """

    def rep4(block):
        out = np.zeros((128, block.shape[1]), np.float32)
        for q in range(NQ):
            out[32 * q : 32 * q + block.shape[0]] = block
        return out

    def taps(conv_w, in_lo, in_hi):
        # [k_in, 128out] per tap, taps concatenated on axis 1
        return np.concatenate(
            [np.asarray(conv_w[:, in_lo:in_hi, d]).T for d in range(KTAP)], axis=1
        ).astype(np.float32)

    def peep_if(wci, wcf):
        # lhsT [32, 128]: out cols 0:32 -> i rows, 32:64 -> f rows, rest 0
        blk = np.zeros((HID, 128), np.float32)
        blk[:, 0:HID] = np.asarray(wci).T
        blk[:, HID : 2 * HID] = np.asarray(wcf).T
        return blk

    # x im2col block [128, 128]: group q rows 3*s + c = tap d, channel c,
    # slot s = d (half 0) or 2-d (half 1) so the edge tap sits at the
    # 32-aligned group base (BIR verifier requires aligned partition bases)
    w0x = np.zeros((128, 128), np.float32)
    for q in range(NQ):
        half = q & 1
        for dd in range(KTAP):
            s = dd if half == 0 else 2 - dd
            w0x[32 * q + 3 * s : 32 * q + 3 * s + CIN, :] = np.asarray(
                conv_w0[:, 0:CIN, dd]).T
    d = dict(
        w0h=rep4(taps(conv_w0, CIN, CIN + HID)),
        w0x=w0x,
        wp0=rep4(peep_if(wci0, wcf0)),
        wpo0=rep4(np.asarray(wco0).T.astype(np.float32)),
        b0=np.asarray(conv_b0, np.float32).reshape(128, 1),
        w1h=rep4(
            np.concatenate([taps(conv_w1, 0, HID), taps(conv_w1, HID, 2 * HID)], 1)
        ),
        wp1=rep4(peep_if(wci1, wcf1)),
        wpo1=rep4(np.asarray(wco1).T.astype(np.float32)),
        b1=np.asarray(conv_b1, np.float32).reshape(128, 1),
    )
    return {k: (v if k.startswith("b") else v.astype(BF)) for k, v in d.items()}


def _build_kernel():
    WH = W // 2
    MMN = 512  # fp32 matmul free-dim cap (one PSUM bank)
    n_chunks = WH // MMN

    nc = bacc.Bacc("TRN2", target_bir_lowering=False, debug=False)

    x_d = nc.dram_tensor("x", [B_SHARD, T, CIN, W], BF16, kind="ExternalInput")
    y_d = nc.dram_tensor("y", [B_SHARD, T, HID, W], BF16, kind="ExternalOutput")
    w_d = {
        name: nc.dram_tensor(
            name, list(shape), F32 if name.startswith("b") else BF16,
            kind="ExternalInput")
        for name, shape in WEIGHT_SHAPES.items()
    }
    x_ap = x_d.ap()
    y_ap = y_d.ap()

    with tile.TileContext(nc) as tc, ExitStack() as ctx:
        const = ctx.enter_context(tc.tile_pool(name="const", bufs=1))
        xpool = ctx.enter_context(tc.tile_pool(name="xp", bufs=3))
        gates = ctx.enter_context(tc.tile_pool(name="gates", bufs=2))
        state = ctx.enter_context(tc.tile_pool(name="state", bufs=1))
        psum = ctx.enter_context(tc.tile_pool(name="psum", bufs=2, space="PSUM"))

        wt = {}
        for name, shape in WEIGHT_SHAPES.items():
            wtile = const.tile(
                list(shape), F32 if name.startswith("b") else BF16,
                name=f"wt_{name}", tag=f"wt_{name}")
            nc.sync.dma_start(wtile[:], w_d[name].ap()[:, :])
            wt[name] = wtile

        # persistent state, ping-pong buffers; h tiles carry 1 halo col/side
        h0 = [state.tile([128, WH + 2], BF16, tag=f"h0_{i}", name=f"h0_{i}") for i in range(2)]
        h1 = [state.tile([128, WH + 2], BF16, tag=f"h1_{i}", name=f"h1_{i}") for i in range(2)]
        c0 = [state.tile([128, WH], F32, tag=f"c0_{i}", name=f"c0_{i}") for i in range(2)]
        c1 = [state.tile([128, WH], F32, tag=f"c1_{i}", name=f"c1_{i}") for i in range(2)]
        # bf16 copies of c for PE rhs (peephole matmuls); fp32 master for DVE/ACT
        cb0 = [state.tile([128, WH], BF16, tag=f"cb0_{i}", name=f"cb0_{i}") for i in range(2)]
        cb1 = [state.tile([128, WH], BF16, tag=f"cb1_{i}", name=f"cb1_{i}") for i in range(2)]
        for tl in (*h0, *h1, *c0, *c1, *cb0, *cb1):
            nc.vector.memset(tl[:], 0.0)

        def halo_fix(h):
            nc.vector.tensor_copy(h[32:64, 0:1], h[0:32, WH : WH + 1])
            nc.vector.tensor_copy(h[0:32, WH + 1 : WH + 2], h[32:64, 1:2])
            nc.vector.tensor_copy(h[96:128, 0:1], h[64:96, WH : WH + 1])
            nc.vector.tensor_copy(h[64:96, WH + 1 : WH + 2], h[96:128, 1:2])

        def load_x(t):
            # im2col over taps: rows 32q + 3d + c = x[b, t, c, w + d - 1]
            # (one K=9 matmul per chunk-quarter instead of 3 K=3 matmuls)
            xt = xpool.tile([128, WH], BF16, tag="xt", name="xt")
            for q in range(NQ):
                b, half = q >> 1, q & 1
                w0 = half * WH
                for d in range(KTAP):
                    s = d if half == 0 else 2 - d
                    lo = w0 + d - 1
                    hi = lo + WH
                    slo, shi = max(lo, 0), min(hi, W)
                    dlo = slo - lo
                    dhi = WH - (hi - shi)
                    rows = slice(32 * q + 3 * s, 32 * q + 3 * s + CIN)
                    nc.sync.dma_start(xt[rows, dlo:dhi], x_ap[b, t, 0:CIN, slo:shi])
                # edge tap sits at slot 0 -> 32-aligned memset base
                erows = slice(32 * q, 32 * q + CIN)
                if half == 0:
                    nc.vector.memset(xt[erows, 0:1], 0.0)
                else:
                    nc.vector.memset(xt[erows, WH - 1 : WH], 0.0)
            return xt

        def step_layer(lyr, xt, h_prev, h_next, c_prev, c_next, cb_prev, cb_next, y_src):
            w_h = wt["w0h"] if lyr == 0 else wt["w1h"]
            w_p = wt["wp0"] if lyr == 0 else wt["wp1"]
            w_po = wt["wpo0"] if lyr == 0 else wt["wpo1"]
            b_t = wt["b0"] if lyr == 0 else wt["b1"]

            s_i = gates.tile([128, WH], F32, tag="s_i", name="s_i")
            s_f = gates.tile([128, WH], F32, tag="s_f", name="s_f")
            t_g = gates.tile([128, WH], F32, tag="t_g", name="t_g")
            o_pre = gates.tile([128, WH], F32, tag="o_pre", name="o_pre")
            s_o = gates.tile([128, WH], F32, tag="s_o", name="s_o", bufs=1)
            t_c = gates.tile([128, WH], F32, tag="t_c", name="t_c", bufs=1)
            m1 = gates.tile([128, WH], F32, tag="m1", name="m1", bufs=1)
            m2 = gates.tile([128, WH], F32, tag="m2", name="m2", bufs=1)

            for q in range(NQ):
                rq = slice(32 * q, 32 * q + 32)
                pq = psum.tile([128, WH], F32, tag="P", name="pq")
                for ch in range(n_chunks):
                    c_lo, c_hi = ch * MMN, (ch + 1) * MMN
                    cs = slice(c_lo, c_hi)
                    first = True
                    if lyr == 0:
                        # x: single K=9 im2col matmul (taps pre-shifted by DMA)
                        nc.tensor.matmul(
                            pq[:, cs],
                            wt["w0x"][32 * q : 32 * q + 3 * KTAP, 0:128],
                            xt[32 * q : 32 * q + 3 * KTAP, c_lo:c_hi],
                            start=True,
                            stop=False,
                            tile_position=(32 * q, 0),
                        )
                        first = False
                        srcs = [(h_prev, w_h, 0, HID)]
                    else:
                        srcs = [(y_src, w_h, 0, HID), (h_prev, w_h, 384, HID)]
                    for src, w_src, wcol, kdim in srcs:
                        for d in range(KTAP):
                            nc.tensor.matmul(
                                pq[:, cs],
                                w_src[32 * q : 32 * q + kdim,
                                      wcol + 128 * d : wcol + 128 * d + 128],
                                src[32 * q : 32 * q + kdim, c_lo + d : c_hi + d],
                                start=first,
                                stop=False,
                                tile_position=(32 * q, 0),
                            )
                            first = False
                    nc.tensor.matmul(
                        pq[:, cs],
                        w_p[rq, :],
                        cb_prev[rq, cs],
                        start=False,
                        stop=True,
                        tile_position=(32 * q, 0),
                    )

                nc.scalar.activation(s_i[rq, :], pq[0:32, :], AF.Sigmoid, bias=b_t[0:32, :])
                nc.scalar.activation(s_f[rq, :], pq[32:64, :], AF.Sigmoid, bias=b_t[32:64, :])
                nc.scalar.activation(t_g[rq, :], pq[96:128, :], AF.Tanh, bias=b_t[96:128, :])
                nc.scalar.activation(o_pre[rq, :], pq[64:96, :], AF.Identity, bias=b_t[64:96, :])

            nc.vector.tensor_mul(m1[:], s_f[:], c_prev[:])
            nc.vector.tensor_mul(m2[:], s_i[:], t_g[:])
            nc.vector.tensor_add(c_next[:], m1[:], m2[:])
            nc.vector.tensor_copy(cb_next[:], c_next[:])

            # peephole o (diagonal tiles, q-packed out, shares "P" slots)
            po = psum.tile([128, WH], F32, tag="P", name="po")
            for q in range(NQ):
                rq = slice(32 * q, 32 * q + 32)
                for ch in range(n_chunks):
                    cs = slice(ch * MMN, (ch + 1) * MMN)
                    nc.tensor.matmul(
                        po[rq, cs],
                        w_po[rq, :],
                        cb_next[rq, cs],
                        start=True,
                        stop=True,
                        tile_position=(32 * q, 32 * q),
                    )
            nc.vector.tensor_add(o_pre[:], o_pre[:], po[:])

            nc.scalar.activation(s_o[:], o_pre[:], AF.Sigmoid)
            nc.scalar.activation(t_c[:], c_next[:], AF.Tanh)
            nc.vector.tensor_mul(h_next[:, 1 : WH + 1], s_o[:], t_c[:])
            halo_fix(h_next)

        for t in range(T):
            cur, nxt = t % 2, (t + 1) % 2
            xt = load_x(t)
            step_layer(0, xt, h0[cur], h0[nxt], c0[cur], c0[nxt], cb0[cur], cb0[nxt], None)
            step_layer(1, None, h1[cur], h1[nxt], c1[cur], c1[nxt], cb1[cur], cb1[nxt], h0[nxt])
            for q in range(NQ):
                b, half = q >> 1, q & 1
                w0 = half * WH
                nc.sync.dma_start(
                    y_ap[b, t, 0:HID, w0 : w0 + WH],
                    h1[nxt][32 * q : 32 * q + 32, 1 : WH + 1],
                )

    nc.compile()
    return nc


_NC_CACHE = None


def _get_nc():
    global _NC_CACHE
    if _NC_CACHE is None:
        _NC_CACHE = _build_kernel()
    return _NC_CACHE


def kernel(x, conv_w0, conv_b0, wci0, wcf0, wco0,
           conv_w1, conv_b1, wci1, wcf1, wco1):
    x = np.ascontiguousarray(np.asarray(x, np.float32).astype(BF))
    packed = _pack_weights(conv_w0, conv_b0, wci0, wcf0, wco0,
                           conv_w1, conv_b1, wci1, wcf1, wco1)
    nc = _get_nc()
    in_maps = []
    for core in range(N_CORES):
        m = {"x": np.ascontiguousarray(x[B_SHARD * core : B_SHARD * (core + 1)])}
        m.update(packed)
        in_maps.append(m)
    res = run_bass_kernel_spmd(nc, in_maps, core_ids=list(range(N_CORES)))
    return np.concatenate(
        [np.asarray(r["y"]).astype(np.float32) for r in res.results], axis=0)



# revision 14
# speedup vs baseline: 1847.2214x; 1.2809x over previous
"""Fused 2-layer peephole ConvLSTM for TRN2 (Bass/Tile), 8-core SPMD.

Problem: x[B=16, T=25, CIN=3, W=4096] -> y[B, T, HID=32, W]; two stacked
ConvLSTM layers (k=3 SAME conv over W, peephole connections), zero-init
states, scanned over T.

Sharding: data-parallel over batch. B=16 -> 2 batches per core on 8 cores;
weights replicated; no collectives. The full recurrence runs on-chip: only
x is read from HBM and only y (layer-1 hidden states) is written back.

Per-core layout ("q-packing"): per-step logical tensors [32ch, 2*W] are
stored as [128, WH] tiles (WH = W/2), partition block 32q:32q+32 holding
quarter q = (batch q>>1, w-half q&1). Every elementwise/activation op over
a step then runs as a single [128, WH] instruction (engine cost tracks the
free-dim length, independent of partition count), and the PE runs the four
quarters' conv matmuls on four different row-groups via tile_position.

Per step, per layer:
  PE : per quarter q (row-group q): 3 h-tap matmuls (K=32, M=128) [+ 3
       x/y-source tap matmuls] + peephole-i/f matmul (wci/wcf padded to
       M=128), accumulated in a PSUM tile [128, WH] with gate rows
       i 0:32 | f 32:64 | o 64:96 | g 96:128.
  ACT: per quarter: sigmoid(i), sigmoid(f), tanh(g), Identity+bias(o),
       each [32, WH] PSUM -> q-packed SBUF.
  DVE: m1 = s_f*c ; m2 = s_i*tanh_g ; c' = m1+m2     (single [128, WH] ops)
  PE : peephole-o: 4 diagonal tile_position matmuls wco.T @ c' -> PSUM
  DVE: o_pre += peep_o ; ACT: s_o = sigmoid(o_pre), t_c = tanh(c')
  DVE: h' = s_o * t_c -> h tile core cols + 4 halo-column copies
  DMA: (layer 1) h' -> y HBM
"""

import sys

for _p in ("/opt/trn_rl_repo",):
    if _p not in sys.path:
        sys.path.insert(0, _p)

from contextlib import ExitStack

import numpy as np

import concourse.bass as bass  # noqa: F401  (bass types used via tile/bacc)
import concourse.tile as tile
from concourse import bacc, mybir
from concourse.bass_utils import run_bass_kernel_spmd

F32 = mybir.dt.float32
BF16 = mybir.dt.bfloat16
AF = mybir.ActivationFunctionType

import ml_dtypes

BF = ml_dtypes.bfloat16

B, T, CIN, HID, W = 16, 25, 3, 32, 4096
KTAP = 3
N_CORES = 8
B_SHARD = B // N_CORES  # 2
NQ = 4  # quarters: (batch 0/1) x (w-half 0/1)

WEIGHT_SHAPES = dict(
    w0h=(128, 384),
    w0x=(128, 128),
    wp0=(128, 128),
    wpo0=(128, 32),
    b0=(128, 1),
    w1h=(128, 768),
    wp1=(128, 128),
    wpo1=(128, 32),
    b1=(128, 1),
)


def _pack_weights(conv_w0, conv_b0, wci0, wcf0, wco0, conv_w1, conv_b1, wci1, wcf1, wco1):
    """Pack reference weights into the SBUF layouts the kernel expects.

    Each [K, M] block is replicated into the 4 partition quarters (the PE
    row-group q streams its rhs, and loads its lhsT, from partitions 32q).
    """

    def rep4(block):
        out = np.zeros((128, block.shape[1]), np.float32)
        for q in range(NQ):
            out[32 * q : 32 * q + block.shape[0]] = block
        return out

    def taps(conv_w, in_lo, in_hi):
        # [k_in, 128out] per tap, taps concatenated on axis 1
        return np.concatenate(
            [np.asarray(conv_w[:, in_lo:in_hi, d]).T for d in range(KTAP)], axis=1
        ).astype(np.float32)

    def peep_if(wci, wcf):
        # lhsT [32, 128]: out cols 0:32 -> i rows, 32:64 -> f rows, rest 0
        blk = np.zeros((HID, 128), np.float32)
        blk[:, 0:HID] = np.asarray(wci).T
        blk[:, HID : 2 * HID] = np.asarray(wcf).T
        return blk

    # x im2col block [128, 128]: group q rows 3*s + c = tap d, channel c,
    # slot s = d (half 0) or 2-d (half 1) so the edge tap sits at the
    # 32-aligned group base (BIR verifier requires aligned partition bases)
    w0x = np.zeros((128, 128), np.float32)
    for q in range(NQ):
        half = q & 1
        for dd in range(KTAP):
            s = dd if half == 0 else 2 - dd
            w0x[32 * q + 3 * s : 32 * q + 3 * s + CIN, :] = np.asarray(
                conv_w0[:, 0:CIN, dd]).T
    d = dict(
        w0h=rep4(taps(conv_w0, CIN, CIN + HID)),
        w0x=w0x,
        wp0=rep4(peep_if(wci0, wcf0)),
        wpo0=rep4(np.asarray(wco0).T.astype(np.float32)),
        b0=np.asarray(conv_b0, np.float32).reshape(128, 1),
        w1h=rep4(
            np.concatenate([taps(conv_w1, 0, HID), taps(conv_w1, HID, 2 * HID)], 1)
        ),
        wp1=rep4(peep_if(wci1, wcf1)),
        wpo1=rep4(np.asarray(wco1).T.astype(np.float32)),
        b1=np.asarray(conv_b1, np.float32).reshape(128, 1),
    )
    return {k: (v if k.startswith("b") else v.astype(BF)) for k, v in d.items()}


def _build_kernel():
    WH = W // 2
    MMN = 512  # fp32 matmul free-dim cap (one PSUM bank)
    n_chunks = WH // MMN

    nc = bacc.Bacc("TRN2", target_bir_lowering=False, debug=False)

    x_d = nc.dram_tensor("x", [B_SHARD, T, CIN, W], BF16, kind="ExternalInput")
    y_d = nc.dram_tensor("y", [B_SHARD, T, HID, W], BF16, kind="ExternalOutput")
    w_d = {
        name: nc.dram_tensor(
            name, list(shape), F32 if name.startswith("b") else BF16,
            kind="ExternalInput")
        for name, shape in WEIGHT_SHAPES.items()
    }
    x_ap = x_d.ap()
    y_ap = y_d.ap()

    with tile.TileContext(nc) as tc, ExitStack() as ctx:
        const = ctx.enter_context(tc.tile_pool(name="const", bufs=1))
        xpool = ctx.enter_context(tc.tile_pool(name="xp", bufs=3))
        gates = ctx.enter_context(tc.tile_pool(name="gates", bufs=2))
        state = ctx.enter_context(tc.tile_pool(name="state", bufs=1))
        psum = ctx.enter_context(tc.tile_pool(name="psum", bufs=2, space="PSUM"))

        wt = {}
        for name, shape in WEIGHT_SHAPES.items():
            wtile = const.tile(
                list(shape), F32 if name.startswith("b") else BF16,
                name=f"wt_{name}", tag=f"wt_{name}")
            nc.sync.dma_start(wtile[:], w_d[name].ap()[:, :])
            wt[name] = wtile

        # persistent state, ping-pong buffers; h tiles carry 1 halo col/side
        h0 = [state.tile([128, WH + 2], BF16, tag=f"h0_{i}", name=f"h0_{i}") for i in range(2)]
        h1 = [state.tile([128, WH + 2], BF16, tag=f"h1_{i}", name=f"h1_{i}") for i in range(2)]
        c0 = [state.tile([128, WH], F32, tag=f"c0_{i}", name=f"c0_{i}") for i in range(2)]
        c1 = [state.tile([128, WH], F32, tag=f"c1_{i}", name=f"c1_{i}") for i in range(2)]
        # bf16 copies of c for PE rhs (peephole matmuls); fp32 master for DVE/ACT
        cb0 = [state.tile([128, WH], BF16, tag=f"cb0_{i}", name=f"cb0_{i}") for i in range(2)]
        cb1 = [state.tile([128, WH], BF16, tag=f"cb1_{i}", name=f"cb1_{i}") for i in range(2)]
        for tl in (*h0, *h1, *c0, *c1, *cb0, *cb1):
            nc.vector.memset(tl[:], 0.0)

        def halo_fix(h):
            nc.vector.tensor_copy(h[32:64, 0:1], h[0:32, WH : WH + 1])
            nc.vector.tensor_copy(h[0:32, WH + 1 : WH + 2], h[32:64, 1:2])
            nc.vector.tensor_copy(h[96:128, 0:1], h[64:96, WH : WH + 1])
            nc.vector.tensor_copy(h[64:96, WH + 1 : WH + 2], h[96:128, 1:2])

        def load_x(t):
            # im2col over taps: rows 32q + 3d + c = x[b, t, c, w + d - 1]
            # (one K=9 matmul per chunk-quarter instead of 3 K=3 matmuls)
            xt = xpool.tile([128, WH], BF16, tag="xt", name="xt")
            for q in range(NQ):
                b, half = q >> 1, q & 1
                w0 = half * WH
                for d in range(KTAP):
                    s = d if half == 0 else 2 - d
                    lo = w0 + d - 1
                    hi = lo + WH
                    slo, shi = max(lo, 0), min(hi, W)
                    dlo = slo - lo
                    dhi = WH - (hi - shi)
                    rows = slice(32 * q + 3 * s, 32 * q + 3 * s + CIN)
                    nc.sync.dma_start(xt[rows, dlo:dhi], x_ap[b, t, 0:CIN, slo:shi])
                # edge tap sits at slot 0 -> 32-aligned memset base
                erows = slice(32 * q, 32 * q + CIN)
                if half == 0:
                    nc.vector.memset(xt[erows, 0:1], 0.0)
                else:
                    nc.vector.memset(xt[erows, WH - 1 : WH], 0.0)
            return xt

        def step_layer(lyr, xt, h_prev, h_next, c_prev, c_next, cb_prev, cb_next, y_src):
            w_h = wt["w0h"] if lyr == 0 else wt["w1h"]
            w_p = wt["wp0"] if lyr == 0 else wt["wp1"]
            w_po = wt["wpo0"] if lyr == 0 else wt["wpo1"]
            b_t = wt["b0"] if lyr == 0 else wt["b1"]

            s_i = gates.tile([128, WH], F32, tag="s_i", name="s_i")
            s_f = gates.tile([128, WH], F32, tag="s_f", name="s_f")
            t_g = gates.tile([128, WH], F32, tag="t_g", name="t_g")
            o_pre = gates.tile([128, WH], F32, tag="o_pre", name="o_pre")
            s_o = gates.tile([128, WH], F32, tag="s_o", name="s_o", bufs=1)
            t_c = gates.tile([128, WH], F32, tag="t_c", name="t_c", bufs=1)
            m1 = gates.tile([128, WH], F32, tag="m1", name="m1", bufs=1)
            m2 = gates.tile([128, WH], F32, tag="m2", name="m2", bufs=1)

            for q in range(NQ):
                rq = slice(32 * q, 32 * q + 32)
                pq = psum.tile([128, WH], F32, tag="P", name="pq")
                for ch in range(n_chunks):
                    c_lo, c_hi = ch * MMN, (ch + 1) * MMN
                    cs = slice(c_lo, c_hi)
                    first = True
                    if lyr == 0:
                        # x: single K=9 im2col matmul (taps pre-shifted by DMA)
                        nc.tensor.matmul(
                            pq[:, cs],
                            wt["w0x"][32 * q : 32 * q + 3 * KTAP, 0:128],
                            xt[32 * q : 32 * q + 3 * KTAP, c_lo:c_hi],
                            start=True,
                            stop=False,
                            tile_position=(32 * q, 0),
                        )
                        first = False
                        srcs = [(h_prev, w_h, 0, HID)]
                    else:
                        srcs = [(y_src, w_h, 0, HID), (h_prev, w_h, 384, HID)]
                    for src, w_src, wcol, kdim in srcs:
                        for d in range(KTAP):
                            nc.tensor.matmul(
                                pq[:, cs],
                                w_src[32 * q : 32 * q + kdim,
                                      wcol + 128 * d : wcol + 128 * d + 128],
                                src[32 * q : 32 * q + kdim, c_lo + d : c_hi + d],
                                start=first,
                                stop=False,
                                tile_position=(32 * q, 0),
                            )
                            first = False
                    nc.tensor.matmul(
                        pq[:, cs],
                        w_p[rq, :],
                        cb_prev[rq, cs],
                        start=False,
                        stop=True,
                        tile_position=(32 * q, 0),
                    )

                nc.scalar.activation(s_i[rq, :], pq[0:32, :], AF.Sigmoid, bias=b_t[0:32, :])
                nc.scalar.activation(s_f[rq, :], pq[32:64, :], AF.Sigmoid, bias=b_t[32:64, :])
                nc.scalar.activation(t_g[rq, :], pq[96:128, :], AF.Tanh, bias=b_t[96:128, :])
                nc.scalar.activation(o_pre[rq, :], pq[64:96, :], AF.Identity, bias=b_t[64:96, :])

            po = psum.tile([128, WH], F32, tag="P", name="po")
            HH = WH // 4
            for hh in range(4):
                hs = slice(hh * HH, (hh + 1) * HH)
                nc.vector.tensor_mul(m1[:, hs], s_f[:, hs], c_prev[:, hs])
                nc.gpsimd.tensor_mul(m2[:, hs], s_i[:, hs], t_g[:, hs])
                nc.vector.tensor_add(c_next[:, hs], m1[:, hs], m2[:, hs])
                nc.gpsimd.tensor_copy(cb_next[:, hs], c_next[:, hs])
                for q in range(NQ):
                    rq = slice(32 * q, 32 * q + 32)
                    for ch in range(hh * n_chunks // 4, (hh + 1) * n_chunks // 4):
                        cs = slice(ch * MMN, (ch + 1) * MMN)
                        nc.tensor.matmul(
                            po[rq, cs],
                            w_po[rq, :],
                            cb_next[rq, cs],
                            start=True,
                            stop=True,
                            tile_position=(32 * q, 32 * q),
                        )
                nc.vector.tensor_add(o_pre[:, hs], o_pre[:, hs], po[:, hs])
                nc.scalar.activation(s_o[:, hs], o_pre[:, hs], AF.Sigmoid)
                nc.scalar.activation(t_c[:, hs], c_next[:, hs], AF.Tanh)
                nc.vector.tensor_mul(
                    h_next[:, 1 + hh * HH : 1 + (hh + 1) * HH], s_o[:, hs], t_c[:, hs])
            halo_fix(h_next)

        for t in range(T):
            cur, nxt = t % 2, (t + 1) % 2
            xt = load_x(t)
            step_layer(0, xt, h0[cur], h0[nxt], c0[cur], c0[nxt], cb0[cur], cb0[nxt], None)
            step_layer(1, None, h1[cur], h1[nxt], c1[cur], c1[nxt], cb1[cur], cb1[nxt], h0[nxt])
            for q in range(NQ):
                b, half = q >> 1, q & 1
                w0 = half * WH
                nc.sync.dma_start(
                    y_ap[b, t, 0:HID, w0 : w0 + WH],
                    h1[nxt][32 * q : 32 * q + 32, 1 : WH + 1],
                )

    nc.compile()
    return nc


_NC_CACHE = None


def _get_nc():
    global _NC_CACHE
    if _NC_CACHE is None:
        _NC_CACHE = _build_kernel()
    return _NC_CACHE


def kernel(x, conv_w0, conv_b0, wci0, wcf0, wco0,
           conv_w1, conv_b1, wci1, wcf1, wco1):
    x = np.ascontiguousarray(np.asarray(x, np.float32).astype(BF))
    packed = _pack_weights(conv_w0, conv_b0, wci0, wcf0, wco0,
                           conv_w1, conv_b1, wci1, wcf1, wco1)
    nc = _get_nc()
    in_maps = []
    for core in range(N_CORES):
        m = {"x": np.ascontiguousarray(x[B_SHARD * core : B_SHARD * (core + 1)])}
        m.update(packed)
        in_maps.append(m)
    res = run_bass_kernel_spmd(nc, in_maps, core_ids=list(range(N_CORES)))
    return np.concatenate(
        [np.asarray(r["y"]).astype(np.float32) for r in res.results], axis=0)

